# revision 9
# baseline (speedup 1.0000x reference)
"""Trainium2 Bass kernel for nn_RandomForest (soft-routed forest, hard decisions).

Algorithm (per sample b, tree t):
  5-level routing with level-local node indices: idx_{L+1} = 2*idx_L + s,
  s = [sigmoid(w_n.x + b_n) <= 0.5] = [w_n.x + b_n <= 0].  Only nodes 0..15 of
  the 31 are ever used (node index at level L is in [0, 2^L)).
  out[b] = mean_t leaves[t, idx5(t,b)].

Device formulation (batch-sharded over 8 cores, all trees on every core):
  MM1 (PE):  logits[t,n,b] = w[t,n,:] . x[b,:]          (dense, 16 nodes/tree)
  ACT:       sigma1 = sign(logits + node_b)             ({-1,+1})
  MM2 (PE):  Ms[t,l,b] = sum_n M[l,n] * sigma1[t,n,b]   (M in {-5..5}, exact)
             leaf l selected  <=>  Ms = -5
  ACT/DVE:   sel: flavorA sign(Ms+4) in {-1:sel,+1:not} / flavorB is_le -> {1,0}
  MM3 (PE):  out[c,b] = sum_{t,l} scaled_leaves[t,l,c] * sel[t,l,b] (+ const)
"""

import numpy as np
import ml_dtypes
from contextlib import ExitStack

import bass_rust
import concourse.bass as bass
import concourse.mybir as mybir
import concourse.tile as tile
from concourse.bass_utils import run_bass_kernel_spmd

# problem constants (hardcoded per harness contract)
T, NN, NL, D, C, B = 100, 16, 32, 256, 64, 8192
NCORES = 8
BC = B // NCORES          # 1024 samples per core
NTILE = 512               # matmul moving free-dim tile
NJ = BC // NTILE          # 2 B-tiles per core
TPAD = 104                # trees padded to 13 blocks of 8
NBLK = TPAD // 8          # 13 blocks  (8 trees x 16 nodes = 128 partitions)
NCHUNK = TPAD // 4        # 26 chunks  (4 trees x 32 leaves = 128 partitions)

MM1_MODE = "bf16x3"       # "f32" | "f32r" | "bf16x3"

_F32 = mybir.dt.float32
_F32R = mybir.dt.float32r
_BF16 = mybir.dt.bfloat16
_BF16NP = ml_dtypes.bfloat16


def _round_f32r(a):
    hi = a.astype(_BF16NP).astype(np.float32)
    lo = (a - hi).astype(_BF16NP).astype(np.float32)
    return hi + lo


def _split_wide_waits(nc, limit=2):
    """Split wide semaphore waits into preceding wait-only instructions
    (TRN2 NX wait-slot limits; varies by instruction struct)."""
    keep_by_type = {"InstDrain": 0, "InstEventSemaphore": 2}
    for fn in nc.m.functions:
        for bb in fn.blocks:
            new_list = []
            for inst in bb.instructions:
                si = inst.sync_info
                waits = list(si.on_wait) if si is not None and si.on_wait else []
                keep = keep_by_type.get(type(inst).__name__, 1)
                if len(waits) > keep:
                    moved = waits[:len(waits) - keep]
                    kept = waits[len(waits) - keep:]
                    chunks = [moved[i:i + limit] for i in range(0, len(moved), limit)]
                    for ci, ch in enumerate(chunks):
                        nop = mybir.InstEventSemaphore(
                            name=f"{inst.name}-wsplit{ci}", ins=[], outs=[])
                        nop.engine = inst.engine
                        nop.sync_info = bass_rust.SyncInfo(on_wait=ch, on_update=[])
                        new_list.append(nop)
                    upd = list(si.on_update) if si.on_update else []
                    inst.sync_info = bass_rust.SyncInfo(on_wait=kept, on_update=upd)
                new_list.append(inst)
            bb.instructions = new_list


def _flavor_a(chunk):
    """True -> sel computed on ACT (sign basis), False -> DVE ({0,1} basis)."""
    return chunk % 2 == 1


def _build_m_matrix():
    """M[l, n]: Ms[l] = sum_L (2*a_L - 1) * sigma[p_L(l)] ; selected <=> Ms=-5."""
    M = np.zeros((NL, NN), dtype=np.float64)
    for leaf in range(NL):
        for L in range(5):
            a = (leaf >> (4 - L)) & 1
            p = leaf >> (5 - L)
            M[leaf, p] += 2 * a - 1
    return M


def _register_const_ap(nc, dtype, value):
    tensor = nc.alloc_sbuf_tensor(f"const-{dtype.name}-{value}", [128, 1], dtype)
    nc.gpsimd.memset(tensor.ap(), value)
    nc.const_aps.aps[(dtype, value)] = tensor.ap()


def _build_bass():
    nc = bass.Bass("TRN2", target_bir_lowering=False, debug=False,
                   num_devices=NCORES)
    _register_const_ap(nc, _F32, 4.0)
    nc.all_engine_barrier()
    wx_dt = {"f32": _F32, "f32r": _F32R, "bf16x3": _BF16}[MM1_MODE]
    nkc = 4 if MM1_MODE == "bf16x3" else 2  # [128,*] K-chunk planes of x/w

    # DRAM I/O
    xt_d = nc.dram_tensor("xt", [nkc, 128, BC], wx_dt, kind="ExternalInput")
    wt_d = nc.dram_tensor("wt", [nkc, 128, TPAD * NN], wx_dt, kind="ExternalInput")
    b_d = nc.dram_tensor("bvec", [128, NBLK], _F32, kind="ExternalInput")
    mp_d = nc.dram_tensor("mprime", [128, 128], _BF16, kind="ExternalInput")
    lt_d = nc.dram_tensor("lt3", [NCHUNK, 128, C], _BF16, kind="ExternalInput")
    kv_d = nc.dram_tensor("kvec", [C, 1], _F32, kind="ExternalInput")
    out_d = nc.dram_tensor("out_t", [C, BC], _F32, kind="ExternalOutput")

    with tile.TileContext(nc) as tc, ExitStack() as ctx:
        const = ctx.enter_context(tc.tile_pool(name="const", bufs=1))
        sig = ctx.enter_context(tc.tile_pool(name="sig", bufs=3))
        outp = ctx.enter_context(tc.tile_pool(name="outp", bufs=1))
        ps_l = ctx.enter_context(tc.tile_pool(name="ps_l", bufs=2, space="PSUM"))
        ps_m = ctx.enter_context(tc.tile_pool(name="ps_m", bufs=2, space="PSUM"))
        ps_o = ctx.enter_context(tc.tile_pool(name="ps_o", bufs=2, space="PSUM"))

        xt_sb = const.tile([128, nkc * BC], wx_dt, tag="xt")
        wt_sb = const.tile([128, nkc * TPAD * NN], wx_dt, tag="wt")
        b_sb = const.tile([128, NBLK], _F32, tag="bv")
        mp_sb = const.tile([128, 128], _BF16, tag="mp")
        lt_sb = const.tile([128, NCHUNK * C], _BF16, tag="lt")
        kv_sb = const.tile([C, 1], _F32, tag="kv")
        for kc in range(nkc):
            nc.sync.dma_start(xt_sb[:, kc * BC:(kc + 1) * BC], xt_d.ap()[kc])
            nc.sync.dma_start(
                wt_sb[:, kc * TPAD * NN:(kc + 1) * TPAD * NN], wt_d.ap()[kc])
        nc.sync.dma_start(b_sb[:], b_d.ap())
        nc.sync.dma_start(mp_sb[:], mp_d.ap())
        for ch in range(NCHUNK):
            nc.sync.dma_start(lt_sb[:, ch * C:(ch + 1) * C], lt_d.ap()[ch])
        nc.sync.dma_start(kv_sb[:], kv_d.ap())

        out_sb = outp.tile([C, BC], _F32, tag="osb")

        for j in range(NJ):
            po = ps_o.tile([C, NTILE], _F32, tag="po")
            for blk in range(NBLK):
                # ---- MM1: logits for 8 trees x 16 nodes
                pl = ps_l.tile([128, NTILE], _F32, tag="pl")
                if MM1_MODE == "bf16x3":
                    # planes: 0=hi k0, 1=hi k1, 2=lo k0, 3=lo k1
                    # products hi.hi + hi.lo + lo.hi (lo.lo negligible)
                    prods = [(kc + 2 * wl, kc + 2 * xl)
                             for kc in range(2)
                             for wl, xl in ((0, 0), (0, 1), (1, 0))]
                else:
                    prods = [(0, 0), (1, 1)]
                for nmm, (wp, xp) in enumerate(prods):
                    nc.tensor.matmul(
                        pl[:],
                        wt_sb[:, wp * TPAD * NN + blk * 128:
                              wp * TPAD * NN + (blk + 1) * 128],
                        xt_sb[:, xp * BC + j * NTILE:
                              xp * BC + j * NTILE + NTILE],
                        start=(nmm == 0), stop=(nmm == len(prods) - 1))

                # ---- sigma1 = sign(logits + node_b)  -> bf16 {-1, +1}
                s1 = sig.tile([128, NTILE], _BF16, tag="s1")
                nc.scalar.sign(s1[:], pl[:], bias=b_sb[:, blk:blk + 1])

                # ---- MM2 + selection + MM3, per 4-tree half-block
                for h in range(2):
                    chunk = blk * 2 + h
                    pm = ps_m.tile([128, NTILE], _F32, tag="pm")
                    nc.tensor.matmul(pm[:], mp_sb[h * 64:(h + 1) * 64, :],
                                     s1[h * 64:(h + 1) * 64, :],
                                     start=True, stop=True)
                    s2 = sig.tile([128, NTILE], _BF16, tag="s2")
                    if _flavor_a(chunk):
                        nc.scalar.sign(s2[:], pm[:], bias=4.0)
                    else:
                        nc.vector.tensor_scalar(
                            s2[:], pm[:], -4.0, None, mybir.AluOpType.is_le)
                    nc.tensor.matmul(po[:], lt_sb[:, chunk * C:(chunk + 1) * C],
                                     s2[:],
                                     start=(chunk == 0), stop=(chunk == NCHUNK - 1))

            # ---- out_t[:, j] = po + kvec
            nc.scalar.add(out_sb[:, j * NTILE:(j + 1) * NTILE], po[:],
                          kv_sb[:, 0:1])
        nc.sync.dma_start(out_d.ap(), out_sb[:])

    _split_wide_waits(nc)
    return nc


_CACHE = {}


def _prep_host(x, node_w, node_b, leaves):
    """Build per-core input maps (all slicing/transpose/dtype prep on host)."""
    Mmat = _build_m_matrix()

    # weights: [T,16,D] -> pad trees -> [D, TPAD*16] transposed, K-chunks
    w = np.zeros((TPAD, NN, D), dtype=np.float32)
    w[:T] = node_w[:, :NN, :]
    wt = np.ascontiguousarray(w.transpose(2, 0, 1).reshape(D, TPAD * NN))

    bv = np.ones((TPAD, NN), dtype=np.float32)
    bv[:T] = node_b[:, :NN]
    b_sb = np.ascontiguousarray(bv.reshape(NBLK, 8 * NN).T)  # [128, NBLK]

    # M' lhsT for MM2: [64 (4t x 16n), 128 (4t x 32l)] block-diagonal,
    # duplicated into partitions 64-127 (matmul needs lhsT/rhs same base
    # partition; rhs for the second half-block sits at partitions 64-127).
    mp = np.zeros((64, 128), dtype=np.float64)
    for tq in range(4):
        mp[tq * NN:(tq + 1) * NN, tq * NL:(tq + 1) * NL] = Mmat.T
    mp = np.concatenate([mp, mp], axis=0).astype(_BF16NP)

    # leaves lhsT chunks + constant
    lv = np.zeros((TPAD, NL, C), dtype=np.float64)
    lv[:T] = leaves
    lt3 = np.zeros((NCHUNK, 128, C), dtype=np.float64)
    kvec = np.zeros((C,), dtype=np.float64)
    for chv in range(NCHUNK):
        blkv = lv[4 * chv:4 * (chv + 1)].reshape(128, C)
        if _flavor_a(chv):
            lt3[chv] = -blkv / (2.0 * T)
            kvec += blkv.sum(axis=0) / (2.0 * T)
        else:
            lt3[chv] = blkv / T
    lt3 = lt3.astype(_BF16NP)
    kvec = kvec.astype(np.float32).reshape(C, 1)

    if MM1_MODE == "bf16x3":
        wt_hi = wt.astype(_BF16NP)
        wt_lo = (wt - wt_hi.astype(np.float32)).astype(_BF16NP)
        wt_planes = np.stack([wt_hi[0:128], wt_hi[128:256],
                              wt_lo[0:128], wt_lo[128:256]])
    elif MM1_MODE == "f32r":
        wtr = _round_f32r(wt)
        wt_planes = np.stack([wtr[0:128], wtr[128:256]])
    else:
        wt_planes = np.stack([wt[0:128], wt[128:256]])

    common = {"wt": wt_planes, "bvec": b_sb, "mprime": mp, "lt3": lt3,
              "kvec": kvec}

    in_maps = []
    for core in range(NCORES):
        xs = np.ascontiguousarray(x[core * BC:(core + 1) * BC].T)  # [D, BC]
        if MM1_MODE == "bf16x3":
            x_hi = xs.astype(_BF16NP)
            x_lo = (xs - x_hi.astype(np.float32)).astype(_BF16NP)
            xt = np.stack([x_hi[0:128], x_hi[128:256],
                           x_lo[0:128], x_lo[128:256]])
        elif MM1_MODE == "f32r":
            xr = _round_f32r(xs)
            xt = np.stack([xr[0:128], xr[128:256]])
        else:
            xt = np.stack([xs[0:128], xs[128:256]])
        in_maps.append({"xt": xt, **common})
    return in_maps


def kernel(x, node_w, node_b, leaves):
    x = np.asarray(x, dtype=np.float32)
    node_w = np.asarray(node_w, dtype=np.float32)
    node_b = np.asarray(node_b, dtype=np.float32)
    leaves = np.asarray(leaves, dtype=np.float32)

    if "nc" not in _CACHE:
        _CACHE["nc"] = _build_bass()
    nc = _CACHE["nc"]
    in_maps = _prep_host(x, node_w, node_b, leaves)
    res = run_bass_kernel_spmd(nc, in_maps, list(range(NCORES)))
    _CACHE["last_results"] = res
    out = np.concatenate([r["out_t"].T for r in res.results], axis=0)
    return out.astype(np.float32)
